# revision 5
# baseline (speedup 1.0000x reference)
"""Trainium2 Bass kernel for nn_FC_63960652972583.

Computation (see reference): for each of NSEL=60000 mask-selected nodes,
two blocks x three layers of per-node 16x16 matvec + bias, a weighted-sum
head, mean over (block, layer), returning (out[NSEL], h0[3,NSEL,16],
h1[3,NSEL,16]).

Strategy:
  - Shard all per-node tensors along the node axis across 8 cores
    (15000 nodes/core).  Each mask entry is owned by exactly one core;
    duplicate mask entries are deduplicated (outputs only depend on the
    node), so each core processes its ~5900 unique local node ids.
  - Host packs, per core, one contiguous f32 row of 1856 floats per node:
    [hw0|hw1 (1536) | feats (96) | hb (96) | tw/6 (96) | sum(tb)/6 (1) | pad]
  - Device kernel: dma_gather (SWDGE) pulls 512 rows per call into SBUF
    (one row per partition), DVE does mul + segmented reduce + bias +
    head, one 128x97 store per 128 nodes.
  - Host scatters per-core compacted outputs back to mask positions.
"""

import os
import sys

import numpy as np

for _p in ("/opt/trn_rl_repo",):
    if _p not in sys.path:
        sys.path.insert(0, _p)

L, N, D = 3, 120000, 16
NSEL = 60000
NCORES = 8
NLOC = N // NCORES  # 15000

ROW = 1856  # floats per packed node row (must make ROW*4 % 256 == 0)
HW_OFF, F_OFF, HB_OFF, TW_OFF, TB_OFF = 0, 1536, 1632, 1728, 1824
OUTW = 97  # h0 (48) | h1 (48) | out (1)
CHUNK = 512  # nodes per dma_gather call


def _build_nc(cap: int, chunk: int = CHUNK, reps: int = 1):
    import concourse.bacc as bacc
    import concourse.mybir as mybir
    from concourse.tile import TileContext

    fp32 = mybir.dt.float32
    Alu = mybir.AluOpType

    nc = bacc.Bacc("TRN2")
    packed = nc.declare_dram_parameter("packed", [NLOC, ROW], fp32, isOutput=False)
    idxw = nc.declare_dram_parameter(
        "idxw", [128, cap // 16], mybir.dt.int16, isOutput=False
    )
    outbuf = nc.declare_dram_parameter(
        "outbuf", [reps * cap, OUTW], fp32, isOutput=True
    )

    nchunks = cap // chunk
    jtiles = chunk // 128

    with TileContext(nc) as tc:
        with (
            tc.tile_pool(name="idx", bufs=1) as idxpool,
            tc.tile_pool(name="g", bufs=2) as gpool,
            tc.tile_pool(name="prod", bufs=3) as ppool,
            tc.tile_pool(name="hout", bufs=4) as hpool,
            tc.tile_pool(name="junk", bufs=2) as jpool,
            tc.tile_pool(name="sv", bufs=4) as svpool,
        ):
            idxt = idxpool.tile([128, cap // 16], mybir.dt.int16)
            nc.sync.dma_start(idxt[:], idxw[:, :])

            for rep, c in ((r, c) for r in range(reps) for c in range(nchunks)):
                g = gpool.tile([128, jtiles, ROW], fp32)
                nc.gpsimd.dma_gather(
                    g[:],
                    packed[:, :],
                    idxt[:, c * (chunk // 16) : (c + 1) * (chunk // 16)],
                    chunk,
                    chunk,
                    ROW,
                )
                for j in range(jtiles):
                    w4 = g[:, j, HW_OFF:F_OFF].rearrange(
                        "p (b d e) -> p b d e", d=16, e=16
                    )
                    fbc = (
                        g[:, j, F_OFF:HB_OFF]
                        .rearrange("p (b e) -> p b e", e=16)
                        .unsqueeze(2)
                        .broadcast_to([128, 6, 16, 16])
                    )
                    prod = ppool.tile([128, 1536], fp32)
                    nc.vector.tensor_tensor(
                        prod[:].rearrange("p (b d e) -> p b d e", d=16, e=16),
                        w4,
                        fbc,
                        Alu.mult,
                    )
                    hout = hpool.tile([128, OUTW], fp32)
                    # h_raw = sum_e prod  -> hout[:, 0:96]
                    nc.vector.tensor_reduce(
                        hout[:, 0:96],
                        prod[:].rearrange("p (h e) -> p h e", e=16),
                        mybir.AxisListType.X,
                        Alu.add,
                    )
                    # h += hb
                    nc.vector.tensor_tensor(
                        hout[:, 0:96], hout[:, 0:96], g[:, j, HB_OFF:TW_OFF], Alu.add
                    )
                    # s = sum_bld h * (tw/6)
                    junk = jpool.tile([128, 96], fp32)
                    sv = svpool.tile([128, 1], fp32)
                    nc.vector.scalar_tensor_tensor(
                        junk[:],
                        hout[:, 0:96],
                        0.0,
                        g[:, j, TW_OFF:TB_OFF],
                        Alu.bypass,
                        Alu.mult,
                        accum_out=sv[:],
                    )
                    # out = s + tbsum
                    nc.vector.tensor_tensor(
                        hout[:, 96:97], sv[:], g[:, j, TB_OFF : TB_OFF + 1], Alu.add
                    )
                    r0 = rep * cap + c * chunk + j * 128
                    nc.sync.dma_start(outbuf[r0 : r0 + 128, :], hout[:])
    nc.compile()
    return nc


def _pack_inputs(feat0, hw0, hb0, tw0, tb0, feat1, hw1, hb1, tw1, tb1):
    """Pack per-node data into one f32 row of ROW floats per node."""
    packed = np.zeros((N, ROW), dtype=np.float32)
    packed[:, 0:768] = np.moveaxis(hw0, 1, 0).reshape(N, 768)
    packed[:, 768:1536] = np.moveaxis(hw1, 1, 0).reshape(N, 768)
    packed[:, 1536:1584] = np.moveaxis(feat0, 1, 0).reshape(N, 48)
    packed[:, 1584:1632] = np.moveaxis(feat1, 1, 0).reshape(N, 48)
    packed[:, 1632:1680] = np.moveaxis(hb0, 1, 0).reshape(N, 48)
    packed[:, 1680:1728] = np.moveaxis(hb1, 1, 0).reshape(N, 48)
    packed[:, 1728:1776] = np.moveaxis(tw0, 1, 0).reshape(N, 48) * (1.0 / 6.0)
    packed[:, 1776:1824] = np.moveaxis(tw1, 1, 0).reshape(N, 48) * (1.0 / 6.0)
    packed[:, 1824] = (tb0.sum(axis=0)[:, 0] + tb1.sum(axis=0)[:, 0]) * (1.0 / 6.0)
    return packed


def _wrap_idx(idx_padded: np.ndarray) -> np.ndarray:
    """[cap] int -> [128, cap//16] int16 in the SWDGE gather wrap order."""
    cap = idx_padded.shape[0]
    w = idx_padded.astype(np.int16).reshape(cap // 16, 16).T  # [16, cap//16]
    return np.tile(w, (8, 1)).copy()  # replicated for the 8 Q7 cores


def kernel(feat0, hw0, hb0, tw0, tb0, feat1, hw1, hb1, tw1, tb1, mask):
    packed_full = _pack_inputs(
        feat0, hw0, hb0, tw0, tb0, feat1, hw1, hb1, tw1, tb1
    )

    mask = np.asarray(mask)
    owner = mask // NLOC
    loc = (mask - owner * NLOC).astype(np.int32)

    pos_list, uniq_list, inv_list = [], [], []
    for k in range(NCORES):
        pos_k = np.nonzero(owner == k)[0]
        uniq, inv = np.unique(loc[pos_k], return_inverse=True)
        pos_list.append(pos_k)
        uniq_list.append(uniq)
        inv_list.append(inv)

    maxu = max(len(u) for u in uniq_list)
    cap = max(CHUNK, ((maxu + CHUNK - 1) // CHUNK) * CHUNK)

    in_maps = []
    for k in range(NCORES):
        idx_padded = np.zeros(cap, dtype=np.int32)
        idx_padded[: len(uniq_list[k])] = uniq_list[k]
        in_maps.append(
            {
                "packed": packed_full[k * NLOC : (k + 1) * NLOC],
                "idxw": _wrap_idx(idx_padded),
            }
        )

    from concourse.bass_utils import run_bass_kernel_spmd

    nc = _build_nc(cap)
    res = run_bass_kernel_spmd(nc, in_maps, list(range(NCORES)))
    global LAST_RESULT
    LAST_RESULT = res
    results = res.results

    out_full = np.zeros(NSEL, dtype=np.float32)
    h0_full = np.zeros((L, NSEL, D), dtype=np.float32)
    h1_full = np.zeros((L, NSEL, D), dtype=np.float32)
    for k in range(NCORES):
        ob = results[k]["outbuf"]
        rows = ob[inv_list[k]]  # [len(pos_k), 97]
        pos_k = pos_list[k]
        out_full[pos_k] = rows[:, 96]
        h0_full[:, pos_k, :] = rows[:, 0:48].reshape(-1, 3, 16).transpose(1, 0, 2)
        h1_full[:, pos_k, :] = rows[:, 48:96].reshape(-1, 3, 16).transpose(1, 0, 2)
    return out_full, h0_full, h1_full


if __name__ == "__main__":
    pass
